# revision 48
# baseline (speedup 1.0000x reference)
"""DotAttention (cross-attention with src mask) Trainium2 Bass kernel.

Problem: B=64, S=8192, H=512 fp32
  scores[b,s] = sum_h enc[b,s,h] * dec[b,h]
  weights     = softmax(where(mask, scores, -inf), axis=s)
  context[b,h]= sum_s weights[b,s] * enc[b,s,h]

Sharding: batch data-parallel across 8 NeuronCores (8 batches/core).
Single streaming pass over encoder_outputs (the 1 GiB dominant input):
  - scores: fused VectorE tensor_tensor_reduce (mult + reduce, mask folded
    in as the reduction's initial value: 0 keep / -30000 masked)
  - p = exp(scores - SHIFT) on ScalarE (softmax is shift invariant; a fixed
    shift avoids a second pass over enc for the max)
  - context: TensorE matmul in float32r (fp22 multiply, fp32 accumulate)
    with p-columns stationary, enc tiles moving, accumulated in PSUM
  - normalization by 1/sum(p) applied to both outputs at the end
"""

import sys

sys.path.insert(0, "/opt/trn_rl_repo")

import numpy as np

_B, _S, _H = 64, 8192, 512
_NCORES = 8
_BL = _B // _NCORES  # batches per core
_P = 128             # s rows per tile (SBUF partitions)
_T = _S // _P        # 64 s-tiles per batch
_TG = 8              # tiles per DMA group (2 MiB per group DMA)
_G = _T // _TG       # 8 groups per batch
_SHIFT = 40.0        # exp shift; scores ~ N(0, sqrt(512)), |max| < ~110
_MASK_NEG = -30000.0

_nc_cache = None


def _build(_BL=_BL, _S=_S):
    import concourse.tile as tile
    from concourse import bacc, mybir

    _T = _S // _P
    _G = _T // _TG

    f32 = mybir.dt.float32
    f32r = mybir.dt.float32r
    AF = mybir.ActivationFunctionType
    ALU = mybir.AluOpType

    nc = bacc.Bacc()
    enc = nc.declare_dram_parameter("enc", [_BL, _S, _H], f32, isOutput=False)
    dec = nc.declare_dram_parameter("dec_in", [_BL, _H], f32, isOutput=False)
    mask = nc.declare_dram_parameter("maskp", [_BL, _P, _T], f32, isOutput=False)
    ctx_out = nc.declare_dram_parameter("ctx_out", [_BL * _H], f32, isOutput=True)
    w_out = nc.declare_dram_parameter("w_out", [_BL, _P, _T], f32, isOutput=True)

    with tile.TileContext(nc) as tc:
        with (
            tc.tile_pool(name="singles", bufs=1) as singles,
            tc.tile_pool(name="encp", bufs=5) as encp,
            tc.tile_pool(name="wst", bufs=3) as wst,
            tc.tile_pool(name="psump", bufs=2, space="PSUM") as psump,
        ):
            # Small input loads go on the scalar HWDGE ring so the big enc
            # streams start immediately on the sync ring.
            dec_in = singles.tile([1, _BL * _H], f32)
            nc.scalar.dma_start(
                out=dec_in, in_=dec[:].rearrange("b h -> (b h)")
            )
            maskp_sb = singles.tile([_P, _BL, _T], f32)
            nc.scalar.dma_start(out=maskp_sb, in_=mask[:].rearrange("b p t -> p b t"))

            dec_sb = singles.tile([_P, _BL, _H], f32)
            scores_sb = singles.tile([_P, _BL, _T], f32)
            p_sb = singles.tile([_P, _BL, _T], f32r)
            psums_sb = singles.tile([_P, _BL], f32)
            dummy = singles.tile([_P, 1], f32)
            ones_col = singles.tile([_P, 1], f32)
            nc.vector.memset(ones_col, 1.0)
            ones_row = singles.tile([1, _P], f32)
            nc.vector.memset(ones_row, 1.0)
            inv_row = singles.tile([1, _BL], f32)
            inv_bc = singles.tile([_P, _BL], f32)
            ctx_row = singles.tile([1, _BL * _H], f32)
            ctx_o_row = singles.tile([1, _BL * _H], f32)

            sums_row_ps = psump.tile([1, _BL], f32)
            invb_ps = psump.tile([_P, _BL], f32)
            decb_ps = psump.tile([_P, _H], f32)

            # Broadcast dec rows to all 128 partitions on-device
            # (ones[1,128]^T @ dec_row[1,H] -> [128, H]) instead of
            # shipping a host-replicated 2 MB tensor through HBM.
            for b in range(_BL):
                nc.tensor.matmul(
                    out=decb_ps[:],
                    lhsT=ones_row[:],
                    rhs=dec_in[0:1, b * _H : (b + 1) * _H],
                    start=True,
                    stop=True,
                )
                nc.scalar.copy(dec_sb[:, b, :], decb_ps[:])

            # Absorb the one-time mask load wait into a regular (non-ISA)
            # DVE op so each AMR carries only its enc-DMA wait.
            warm = singles.tile([_P, 1], f32)
            nc.vector.tensor_tensor(
                out=warm[:],
                in0=dec_sb[:, 0, 0:1],
                in1=maskp_sb[:, 0, 0:1],
                op=ALU.mult,
            )

            for b in range(_BL):
                ctx_ps = psump.tile([1, _H], f32, tag="ctxp")
                for g in range(_G):
                    # Partition p holds s = g*1024 + p*8 + r (r = 0..7):
                    # 8 consecutive DRAM rows = one 16 KiB contiguous run
                    # per partition (8x fewer DMA descriptors than an
                    # s = t*128 + p mapping). The s<->(p,r) permutation is
                    # transparent to softmax; host marshals mask/weights
                    # with the same mapping.
                    enc_g = encp.tile([_P, _TG, _H], f32r)
                    src = enc[b, g * _TG * _P : (g + 1) * _TG * _P, :].rearrange(
                        "(p r) h -> p r h", r=_TG
                    )
                    if b == 0 and g < 2:
                        # Concurrent in-flight loads complete near-together
                        # (SDMA round-robins between queues), so the very
                        # first tile would otherwise only land after several
                        # whole groups of bytes. Split the first groups into
                        # per-tile DMAs so compute starts within a few us.
                        for r in range(_TG):
                            nc.sync.dma_start(
                                out=enc_g[:, r, :],
                                in_=src[:, r, :].bitcast(f32r),
                            )
                    else:
                        # Two 1 MiB halves (8 KiB contiguous per partition):
                        # finer arrival granularity smooths the compute pace.
                        h2 = _TG // 2
                        nc.sync.dma_start(
                            out=enc_g[:, :h2, :], in_=src[:, :h2, :].bitcast(f32r)
                        )
                        nc.sync.dma_start(
                            out=enc_g[:, h2:, :], in_=src[:, h2:, :].bitcast(f32r)
                        )
                    for t in range(_TG):
                        col = g * _TG + t
                        nc.vector.affine_mul_reduce(
                            out=dummy.broadcast_to((_P, _H)),
                            accum_out=scores_sb[:, b, col : col + 1],
                            in0=enc_g[:, t, :].bitcast(f32),
                            in1=dec_sb[:, b, :],
                            scale=1.0,
                            bias=0.0,
                        )
                        nc.scalar.activation(
                            out=p_sb[:, b, col : col + 1],
                            in_=scores_sb[:, b, col : col + 1],
                            func=AF.Exp,
                            bias=maskp_sb[:, b, col : col + 1],
                            scale=1.0,
                        )
                    for t in range(_TG):
                        col = g * _TG + t
                        nc.tensor.matmul(
                            out=ctx_ps[:],
                            lhsT=p_sb[:, b, col : col + 1],
                            rhs=enc_g[:, t, :],
                            start=(col == 0),
                            stop=(col == _T - 1),
                        )
                nc.scalar.copy(ctx_row[0:1, b * _H : (b + 1) * _H], ctx_ps[:])
                nc.vector.tensor_reduce(
                    out=psums_sb[:, b : b + 1],
                    in_=p_sb[:, b, :].bitcast(f32),
                    axis=mybir.AxisListType.X,
                    op=ALU.add,
                )

                # Per-batch finale, overlapped with the next batch's stream:
                # cross-partition sum, reciprocal, broadcast, normalize, and
                # write this batch's outputs.
                nc.tensor.matmul(
                    out=sums_row_ps[0:1, b : b + 1],
                    lhsT=ones_col[:],
                    rhs=psums_sb[:, b : b + 1],
                    start=True,
                    stop=True,
                )
                nc.vector.reciprocal(
                    inv_row[0:1, b : b + 1], sums_row_ps[0:1, b : b + 1]
                )
                nc.tensor.matmul(
                    out=invb_ps[:, b : b + 1],
                    lhsT=ones_row[:],
                    rhs=inv_row[0:1, b : b + 1],
                    start=True,
                    stop=True,
                )
                nc.scalar.copy(inv_bc[:, b : b + 1], invb_ps[:, b : b + 1])
                w_st = wst.tile([_P, _T], f32)
                nc.scalar.mul(
                    w_st[:], p_sb[:, b, :].bitcast(f32), inv_bc[:, b : b + 1]
                )
                nc.scalar.dma_start(out=w_out[b], in_=w_st[:])
                nc.scalar.mul(
                    ctx_o_row[0:1, b * _H : (b + 1) * _H],
                    ctx_row[0:1, b * _H : (b + 1) * _H],
                    inv_row[0:1, b : b + 1],
                )
                nc.scalar.dma_start(
                    out=ctx_out[b * _H : (b + 1) * _H],
                    in_=ctx_o_row[0:1, b * _H : (b + 1) * _H],
                )

    nc.compile()
    return nc


def _get_nc():
    global _nc_cache
    if _nc_cache is None:
        _nc_cache = _build()
    return _nc_cache


def kernel(decoder_hidden, encoder_outputs, src_mask):
    from concourse.bass_utils import run_bass_kernel_spmd

    decoder_hidden = np.asarray(decoder_hidden, dtype=np.float32)
    encoder_outputs = np.asarray(encoder_outputs, dtype=np.float32)
    src_mask = np.asarray(src_mask)

    nc = _get_nc()

    in_maps = []
    for c in range(_NCORES):
        sl = slice(c * _BL, (c + 1) * _BL)
        enc_c = np.ascontiguousarray(encoder_outputs[sl])
        dec_c = np.ascontiguousarray(decoder_hidden[sl])
        # maskp[b, p, g*8+r]: exp bias (mask + shift) for s = g*1024+p*8+r
        m = (
            src_mask[sl]
            .reshape(_BL, _G, _P, _TG)
            .transpose(0, 2, 1, 3)
            .reshape(_BL, _P, _T)
        )
        mask_c = np.where(m, -_SHIFT, _MASK_NEG - _SHIFT).astype(np.float32)
        mask_c = np.ascontiguousarray(mask_c)
        in_maps.append({"enc": enc_c, "dec_in": dec_c, "maskp": mask_c})

    res = run_bass_kernel_spmd(nc, in_maps, list(range(_NCORES)))

    ctx = np.concatenate(
        [r["ctx_out"].reshape(_BL, _H) for r in res.results], axis=0
    )
    w_t = np.concatenate([r["w_out"] for r in res.results], axis=0)
    # w_t[b, p, g*8+r] -> weights[b, g*1024 + p*8 + r]
    weights = np.ascontiguousarray(
        w_t.reshape(_B, _P, _G, _TG).transpose(0, 2, 1, 3)
    ).reshape(_B, _S)
    return ctx, weights


# revision 49
# speedup vs baseline: 1.0406x; 1.0406x over previous
"""DotAttention (cross-attention with src mask) Trainium2 Bass kernel.

Problem: B=64, S=8192, H=512 fp32
  scores[b,s] = sum_h enc[b,s,h] * dec[b,h]
  weights     = softmax(where(mask, scores, -inf), axis=s)
  context[b,h]= sum_s weights[b,s] * enc[b,s,h]

Sharding: batch data-parallel across 8 NeuronCores (8 batches/core).
Single streaming pass over encoder_outputs (the 1 GiB dominant input):
  - scores: fused VectorE tensor_tensor_reduce (mult + reduce, mask folded
    in as the reduction's initial value: 0 keep / -30000 masked)
  - p = exp(scores - SHIFT) on ScalarE (softmax is shift invariant; a fixed
    shift avoids a second pass over enc for the max)
  - context: TensorE matmul in float32r (fp22 multiply, fp32 accumulate)
    with p-columns stationary, enc tiles moving, accumulated in PSUM
  - normalization by 1/sum(p) applied to both outputs at the end
"""

import sys

sys.path.insert(0, "/opt/trn_rl_repo")

import numpy as np

_B, _S, _H = 64, 8192, 512
_NCORES = 8
_BL = _B // _NCORES  # batches per core
_P = 128             # s rows per tile (SBUF partitions)
_T = _S // _P        # 64 s-tiles per batch
_TG = 8              # tiles per DMA group (2 MiB per group DMA)
_G = _T // _TG       # 8 groups per batch
_SHIFT = 40.0        # exp shift; scores ~ N(0, sqrt(512)), |max| < ~110
_MASK_NEG = -30000.0

_nc_cache = None


def _build(_BL=_BL, _S=_S):
    import concourse.tile as tile
    from concourse import bacc, mybir

    _T = _S // _P
    _G = _T // _TG

    f32 = mybir.dt.float32
    f32r = mybir.dt.float32r
    AF = mybir.ActivationFunctionType
    ALU = mybir.AluOpType

    nc = bacc.Bacc()
    enc = nc.declare_dram_parameter("enc", [_BL, _S, _H], f32, isOutput=False)
    dec = nc.declare_dram_parameter("dec_in", [_BL, _H], f32, isOutput=False)
    mask = nc.declare_dram_parameter("maskp", [_BL, _P, _T], f32, isOutput=False)
    ctx_out = nc.declare_dram_parameter("ctx_out", [_BL * _H], f32, isOutput=True)
    w_out = nc.declare_dram_parameter("w_out", [_BL, _P, _T], f32, isOutput=True)

    with tile.TileContext(nc) as tc:
        with (
            tc.tile_pool(name="singles", bufs=1) as singles,
            tc.tile_pool(name="encp", bufs=5) as encp,
            tc.tile_pool(name="wst", bufs=3) as wst,
            tc.tile_pool(name="psump", bufs=2, space="PSUM") as psump,
        ):
            # Small input loads go on the scalar HWDGE ring so the big enc
            # streams start immediately on the sync ring.
            dec_in = singles.tile([1, _BL * _H], f32)
            nc.scalar.dma_start(
                out=dec_in, in_=dec[:].rearrange("b h -> (b h)")
            )
            maskp_sb = singles.tile([_P, _BL, _T], f32)
            nc.scalar.dma_start(out=maskp_sb, in_=mask[:].rearrange("b p t -> p b t"))

            dec_sb = singles.tile([_P, _BL, _H], f32)
            scores_sb = singles.tile([_P, _BL, _T], f32)
            p_sb = singles.tile([_P, _BL, _T], f32r)
            psums_sb = singles.tile([_P, _BL], f32)
            dummy = singles.tile([_P, 1], f32)
            ones_col = singles.tile([_P, 1], f32)
            nc.vector.memset(ones_col, 1.0)
            ones_row = singles.tile([1, _P], f32)
            nc.vector.memset(ones_row, 1.0)
            inv_row = singles.tile([1, _BL], f32)
            inv_bc = singles.tile([_P, _BL], f32)
            ctx_row = singles.tile([1, _BL * _H], f32)
            ctx_o_row = singles.tile([1, _BL * _H], f32)

            sums_row_ps = psump.tile([1, _BL], f32)
            invb_ps = psump.tile([_P, _BL], f32)
            decb_ps = psump.tile([_P, _H], f32)

            # Broadcast dec rows to all 128 partitions on-device
            # (ones[1,128]^T @ dec_row[1,H] -> [128, H]) instead of
            # shipping a host-replicated 2 MB tensor through HBM.
            for b in range(_BL):
                nc.tensor.matmul(
                    out=decb_ps[:],
                    lhsT=ones_row[:],
                    rhs=dec_in[0:1, b * _H : (b + 1) * _H],
                    start=True,
                    stop=True,
                )
                nc.scalar.copy(dec_sb[:, b, :], decb_ps[:])

            # Absorb the one-time mask load wait into a regular (non-ISA)
            # DVE op so each AMR carries only its enc-DMA wait.
            warm = singles.tile([_P, 1], f32)
            nc.vector.tensor_tensor(
                out=warm[:],
                in0=dec_sb[:, 0, 0:1],
                in1=maskp_sb[:, 0, 0:1],
                op=ALU.mult,
            )

            for b in range(_BL):
                ctx_ps = psump.tile([1, _H], f32, tag="ctxp")
                for g in range(_G):
                    # Partition p holds s = g*1024 + p*8 + r (r = 0..7):
                    # 8 consecutive DRAM rows = one 16 KiB contiguous run
                    # per partition (8x fewer DMA descriptors than an
                    # s = t*128 + p mapping). The s<->(p,r) permutation is
                    # transparent to softmax; host marshals mask/weights
                    # with the same mapping.
                    enc_g = encp.tile([_P, _TG, _H], f32r)
                    src = enc[b, g * _TG * _P : (g + 1) * _TG * _P, :].rearrange(
                        "(p r) h -> p r h", r=_TG
                    )
                    if b == 0 and g < 4:
                        # Concurrent in-flight loads complete near-together
                        # (SDMA round-robins between queues), so the very
                        # first tile would otherwise only land after several
                        # whole groups of bytes. Split the first groups into
                        # per-tile DMAs so compute starts within a few us.
                        for r in range(_TG):
                            nc.sync.dma_start(
                                out=enc_g[:, r, :],
                                in_=src[:, r, :].bitcast(f32r),
                            )
                    else:
                        # Two 1 MiB halves (8 KiB contiguous per partition):
                        # finer arrival granularity smooths the compute pace.
                        h2 = _TG // 2
                        nc.sync.dma_start(
                            out=enc_g[:, :h2, :], in_=src[:, :h2, :].bitcast(f32r)
                        )
                        nc.sync.dma_start(
                            out=enc_g[:, h2:, :], in_=src[:, h2:, :].bitcast(f32r)
                        )
                    for t in range(_TG):
                        col = g * _TG + t
                        nc.vector.affine_mul_reduce(
                            out=dummy.broadcast_to((_P, _H)),
                            accum_out=scores_sb[:, b, col : col + 1],
                            in0=enc_g[:, t, :].bitcast(f32),
                            in1=dec_sb[:, b, :],
                            scale=1.0,
                            bias=0.0,
                        )
                        nc.scalar.activation(
                            out=p_sb[:, b, col : col + 1],
                            in_=scores_sb[:, b, col : col + 1],
                            func=AF.Exp,
                            bias=maskp_sb[:, b, col : col + 1],
                            scale=1.0,
                        )
                    for t in range(_TG):
                        col = g * _TG + t
                        nc.tensor.matmul(
                            out=ctx_ps[:],
                            lhsT=p_sb[:, b, col : col + 1],
                            rhs=enc_g[:, t, :],
                            start=(col == 0),
                            stop=(col == _T - 1),
                        )
                nc.scalar.copy(ctx_row[0:1, b * _H : (b + 1) * _H], ctx_ps[:])
                nc.vector.tensor_reduce(
                    out=psums_sb[:, b : b + 1],
                    in_=p_sb[:, b, :].bitcast(f32),
                    axis=mybir.AxisListType.X,
                    op=ALU.add,
                )

                # Per-batch finale, overlapped with the next batch's stream:
                # cross-partition sum, reciprocal, broadcast, normalize, and
                # write this batch's outputs.
                nc.tensor.matmul(
                    out=sums_row_ps[0:1, b : b + 1],
                    lhsT=ones_col[:],
                    rhs=psums_sb[:, b : b + 1],
                    start=True,
                    stop=True,
                )
                nc.vector.reciprocal(
                    inv_row[0:1, b : b + 1], sums_row_ps[0:1, b : b + 1]
                )
                nc.tensor.matmul(
                    out=invb_ps[:, b : b + 1],
                    lhsT=ones_row[:],
                    rhs=inv_row[0:1, b : b + 1],
                    start=True,
                    stop=True,
                )
                nc.scalar.copy(inv_bc[:, b : b + 1], invb_ps[:, b : b + 1])
                w_st = wst.tile([_P, _T], f32)
                nc.scalar.mul(
                    w_st[:], p_sb[:, b, :].bitcast(f32), inv_bc[:, b : b + 1]
                )
                nc.scalar.dma_start(out=w_out[b], in_=w_st[:])
                nc.scalar.mul(
                    ctx_o_row[0:1, b * _H : (b + 1) * _H],
                    ctx_row[0:1, b * _H : (b + 1) * _H],
                    inv_row[0:1, b : b + 1],
                )
                nc.scalar.dma_start(
                    out=ctx_out[b * _H : (b + 1) * _H],
                    in_=ctx_o_row[0:1, b * _H : (b + 1) * _H],
                )

    nc.compile()
    return nc


def _get_nc():
    global _nc_cache
    if _nc_cache is None:
        _nc_cache = _build()
    return _nc_cache


def kernel(decoder_hidden, encoder_outputs, src_mask):
    from concourse.bass_utils import run_bass_kernel_spmd

    decoder_hidden = np.asarray(decoder_hidden, dtype=np.float32)
    encoder_outputs = np.asarray(encoder_outputs, dtype=np.float32)
    src_mask = np.asarray(src_mask)

    nc = _get_nc()

    in_maps = []
    for c in range(_NCORES):
        sl = slice(c * _BL, (c + 1) * _BL)
        enc_c = np.ascontiguousarray(encoder_outputs[sl])
        dec_c = np.ascontiguousarray(decoder_hidden[sl])
        # maskp[b, p, g*8+r]: exp bias (mask + shift) for s = g*1024+p*8+r
        m = (
            src_mask[sl]
            .reshape(_BL, _G, _P, _TG)
            .transpose(0, 2, 1, 3)
            .reshape(_BL, _P, _T)
        )
        mask_c = np.where(m, -_SHIFT, _MASK_NEG - _SHIFT).astype(np.float32)
        mask_c = np.ascontiguousarray(mask_c)
        in_maps.append({"enc": enc_c, "dec_in": dec_c, "maskp": mask_c})

    res = run_bass_kernel_spmd(nc, in_maps, list(range(_NCORES)))

    ctx = np.concatenate(
        [r["ctx_out"].reshape(_BL, _H) for r in res.results], axis=0
    )
    w_t = np.concatenate([r["w_out"] for r in res.results], axis=0)
    # w_t[b, p, g*8+r] -> weights[b, g*1024 + p*8 + r]
    weights = np.ascontiguousarray(
        w_t.reshape(_B, _P, _G, _TG).transpose(0, 2, 1, 3)
    ).reshape(_B, _S)
    return ctx, weights
